# revision 3
# baseline (speedup 1.0000x reference)
"""DiscreteLSTM Trainium2 kernel — data-parallel over batch, zero collectives.

Reference math per step t:
    inp = h @ Wh + E[x_t] + b
    f,i,o = sigmoid(inp @ W{f,i,o} + b{f,i,o}); c = tanh(inp @ Wc + bc)
    h' = f*h + i*c ; y = o*tanh(h')

Folded form (exact up to fp reassociation):
    pre_g = h @ (Wh @ Wg) + T[x_t]   where T[v] = (E[v]+b) @ Wg + bg
so each step is 1 gate matmul + a row-gather from a per-token gate table.

Sharding: pure data-parallel — core c owns batch rows [c*16,(c+1)*16) and
runs the full recurrence on them with replicated weights. No inter-core
communication at all, so the whole problem is a single NEFF execution.

Layouts (per core, per step):
  pre-gates in PSUM [48, 2048] f32: rows 0:16 = batch for (f|i) col-blocks,
  rows 32:48 = batch for (o|c). This packs the 4 gates into 4 PSUM banks
  (matmul out base-partition must be a multiple of 32) and lets one
  sigmoid cover f,i. Matmul operands are bf16 (4x PE rate): stationary
  lhsT = h^T chunk [128,16], moving rhs = folded weight cols. The
  gathered table rows seed PSUM via a 16x16-identity matmul. h' is
  transposed back to unit-major via 8 PE matmuls against the identity.

The 32000x4096 bf16 gate table is built on-device per core (dense
near-roofline bf16 matmul over the full vocab, ~3.4 ms).
"""

import numpy as np

B = 128
S = 512
UNITS = 1024
VOCAB = 32000
NCORES = 8
NB = B // NCORES           # 16 batch rows per core
KCH = UNITS // 128         # 8 contraction chunks
VCH = VOCAB // 128         # 250 vocab chunks
GN = 4 * UNITS             # 4096 packed gate cols, order (f,i,o,c)

# gate -> (psum partition base, psum col base) in the [48, 2048] gates tile
GATE_POS = {"f": (0, 0), "i": (0, 1024), "o": (32, 0), "c": (32, 1024)}
GATE_COL = {"f": 0, "i": 1024, "o": 2048, "c": 3072}  # col block in table/wq


def _build_chunk(s_steps: int, with_table: bool, with_hin: bool):
    """One NEFF: optionally build the gate table, then run s_steps of the
    recurrence on this core's 16 batch rows. I/O (per core):
      in:  wq [128,KCH*GN] bf16, x [NB,s] i32, i16 [16,16] bf16,
           if with_table: wg [128,KCH*GN] bf16, eT [VCH,128,KCH*128] bf16,
                          bgb [128,GN] f32
           else:          tab_in [VOCAB,GN] bf16
           if with_hin:   h_in [NB,UNITS] f32
      out: y [s,NB,UNITS] f32, h_out [NB,UNITS] f32,
           tab [VOCAB,GN] bf16 (only when with_table)
    """
    import concourse.bass as bass
    import concourse.mybir as mybir
    import concourse.tile as tile
    from concourse import bacc

    f32 = mybir.dt.float32
    bf16 = mybir.dt.bfloat16
    i32 = mybir.dt.int32
    AF = mybir.ActivationFunctionType
    ALU = mybir.AluOpType

    nc = bacc.Bacc(
        "TRN2",
        target_bir_lowering=False,
        debug=False,
        num_devices=NCORES,
        enable_partition_id=False,
    )

    wq = nc.dram_tensor("wq", [128, KCH * GN], bf16, kind="ExternalInput")
    xin = nc.dram_tensor("x", [NB, s_steps], i32, kind="ExternalInput")
    i16in = nc.dram_tensor("i16", [16, 16], bf16, kind="ExternalInput")
    if with_table:
        wg = nc.dram_tensor("wg", [128, KCH * GN], bf16, kind="ExternalInput")
        eT = nc.dram_tensor("eT", [VCH, 128, KCH * 128], bf16, kind="ExternalInput")
        bgb = nc.dram_tensor("bgb", [128, GN], f32, kind="ExternalInput")
        tab = nc.dram_tensor("tab", [VOCAB, GN], bf16, kind="ExternalOutput")
    else:
        tab = nc.dram_tensor("tab_in", [VOCAB, GN], bf16, kind="ExternalInput")
    if with_hin:
        h_in = nc.dram_tensor("h_in", [NB, UNITS], f32, kind="ExternalInput")
    yout = nc.dram_tensor("y", [s_steps, NB, UNITS], f32, kind="ExternalOutput")
    h_out = nc.dram_tensor("h_out", [NB, UNITS], f32, kind="ExternalOutput")

    with tile.TileContext(nc) as tc:
        with (
            tc.tile_pool(name="const", bufs=1) as cpool,
            tc.tile_pool(name="gbuf", bufs=3) as gpool,
            tc.tile_pool(name="htile", bufs=2) as hpool,
        ):
            i16_sb = cpool.tile([16, 16], bf16, name="i16_sb")
            nc.sync.dma_start(i16_sb[:], i16in[:])
            x_sb = cpool.tile([NB, s_steps], i32, name="x_sb")
            nc.sync.dma_start(x_sb[:], xin[:])
            wq_sb = cpool.tile([128, KCH * GN], bf16, name="wq_sb")
            nc.sync.dma_start(wq_sb[:], wq[:])

            if with_table:
                # ---------- phase 1: tab = (E+b) @ WgPack + bg ----------
                with (
                    tc.tile_pool(name="wgp", bufs=1) as wgpool,
                    tc.tile_pool(name="etile", bufs=3) as epool,
                    tc.tile_pool(name="tstage", bufs=3) as tpool,
                    tc.tile_pool(name="psum_t", bufs=2, space="PSUM") as ppt,
                ):
                    wg_sb = wgpool.tile([128, KCH * GN], bf16, name="wg_sb")
                    nc.sync.dma_start(wg_sb[:], wg[:])
                    bgb_sb = wgpool.tile([128, GN], f32, name="bgb_sb")
                    nc.sync.dma_start(bgb_sb[:], bgb[:])
                    for v in range(VCH):
                        et = epool.tile([128, KCH * 128], bf16, name="et")
                        nc.sync.dma_start(et[:], eT[v])
                        for half in range(2):
                            pt = ppt.tile([128, 2048], f32, space="PSUM", name="pt")
                            for k in range(KCH):
                                for q in range(4):
                                    nc.tensor.matmul(
                                        pt[:, q * 512 : (q + 1) * 512],
                                        lhsT=et[:, k * 128 : (k + 1) * 128],
                                        rhs=wg_sb[
                                            :,
                                            k * GN
                                            + half * 2048
                                            + q * 512 : k * GN
                                            + half * 2048
                                            + (q + 1) * 512,
                                        ],
                                        start=(k == 0),
                                        stop=(k == KCH - 1),
                                    )
                            ts = tpool.tile([128, 2048], bf16, name="ts")
                            nc.vector.tensor_tensor(
                                out=ts[:],
                                in0=pt[:],
                                in1=bgb_sb[:, half * 2048 : (half + 1) * 2048],
                                op=ALU.add,
                            )
                            nc.gpsimd.dma_start(
                                tab[
                                    v * 128 : (v + 1) * 128,
                                    half * 2048 : (half + 1) * 2048,
                                ],
                                ts[:],
                            )

            # ---------- phase 2: recurrence ----------
            with (
                tc.tile_pool(name="work", bufs=2) as wpool,
                tc.tile_pool(name="psum_g", bufs=1, space="PSUM") as ppg,
                tc.tile_pool(name="psum_tr", bufs=2, space="PSUM") as ppr,
            ):
                if with_hin:
                    h_prev = wpool.tile([NB, UNITS], f32, name="h_new")
                    nc.sync.dma_start(h_prev[:], h_in[:])
                    hbf = wpool.tile([NB, UNITS], bf16, name="hbf")
                    nc.vector.tensor_copy(hbf[:], h_prev[:])
                    psT0 = ppr.tile([128, 128], f32, space="PSUM", name="psT")
                    for j in range(KCH):
                        nc.tensor.matmul(
                            psT0[:, j * 16 : (j + 1) * 16],
                            lhsT=hbf[:, j * 128 : (j + 1) * 128],
                            rhs=i16_sb[:],
                            start=True,
                            stop=True,
                        )
                    hT = hpool.tile([128, 128], bf16, name="hT")
                    nc.vector.tensor_copy(hT[:], psT0[:])
                else:
                    h_prev = None
                    hT = None

                for t in range(s_steps):
                    g_sb = gpool.tile([NB, GN], bf16, name="g_sb")
                    nc.gpsimd.indirect_dma_start(
                        out=g_sb[:],
                        out_offset=None,
                        in_=tab[:],
                        in_offset=bass.IndirectOffsetOnAxis(
                            ap=x_sb[:, t : t + 1], axis=0
                        ),
                    )

                    first = h_prev is None
                    ps = ppg.tile([48, 2048], f32, space="PSUM", name="ps_gate")
                    # seed each gate region with the gathered table rows
                    for gname in "fioc":
                        pb, cb = GATE_POS[gname]
                        tc_ = GATE_COL[gname]
                        for j in range(2):
                            nc.tensor.matmul(
                                ps[pb : pb + 16, cb + j * 512 : cb + (j + 1) * 512],
                                lhsT=i16_sb[:],
                                rhs=g_sb[:, tc_ + j * 512 : tc_ + (j + 1) * 512],
                                start=True,
                                stop=first,
                            )
                    # accumulate h @ Wq ; order f,i,c then o so the h'-path
                    # activations can start while the PE finishes o
                    if not first:
                        for gname in "fico":
                            pb, cb = GATE_POS[gname]
                            tc_ = GATE_COL[gname]
                            for k in range(KCH):
                                for j in range(2):
                                    nc.tensor.matmul(
                                        ps[
                                            pb : pb + 16,
                                            cb + j * 512 : cb + (j + 1) * 512,
                                        ],
                                        lhsT=hT[:, k * 16 : (k + 1) * 16],
                                        rhs=wq_sb[
                                            :,
                                            k * GN
                                            + tc_
                                            + j * 512 : k * GN
                                            + tc_
                                            + (j + 1) * 512,
                                        ],
                                        start=False,
                                        stop=(k == KCH - 1),
                                    )
                            if gname == "i":
                                sb_fi = wpool.tile([16, 2048], f32, name="sb_fi")
                                nc.scalar.activation(
                                    sb_fi[:], ps[0:16, :], AF.Sigmoid
                                )
                            elif gname == "c":
                                sb_c = wpool.tile([16, 1024], f32, name="sb_c")
                                nc.scalar.activation(
                                    sb_c[:], ps[32:48, 1024:2048], AF.Tanh
                                )
                    else:
                        sb_fi = wpool.tile([16, 2048], f32, name="sb_fi")
                        nc.scalar.activation(sb_fi[:], ps[0:16, :], AF.Sigmoid)
                        sb_c = wpool.tile([16, 1024], f32, name="sb_c")
                        nc.scalar.activation(sb_c[:], ps[32:48, 1024:2048], AF.Tanh)
                    sb_o = wpool.tile([16, 1024], f32, name="sb_o")
                    nc.scalar.activation(sb_o[:], ps[32:48, 0:1024], AF.Sigmoid)

                    # h' = f*h + i*c
                    t2 = wpool.tile([16, 1024], f32, name="t2")
                    nc.vector.tensor_tensor(
                        out=t2[:], in0=sb_fi[:, 1024:2048], in1=sb_c[:], op=ALU.mult
                    )
                    if first:
                        h_new = t2
                    else:
                        t1 = wpool.tile([16, 1024], f32, name="t1")
                        nc.vector.tensor_tensor(
                            out=t1[:], in0=sb_fi[:, 0:1024], in1=h_prev[:],
                            op=ALU.mult,
                        )
                        h_new = wpool.tile([16, 1024], f32, name="h_new")
                        nc.vector.tensor_tensor(
                            out=h_new[:], in0=t1[:], in1=t2[:], op=ALU.add
                        )

                    if t < s_steps - 1:
                        # transpose h' to unit-major bf16 for the next matmul
                        hbf = wpool.tile([16, 1024], bf16, name="hbf")
                        nc.vector.tensor_copy(hbf[:], h_new[:])
                        psT = ppr.tile([128, 128], f32, space="PSUM", name="psT")
                        for j in range(KCH):
                            nc.tensor.matmul(
                                psT[:, j * 16 : (j + 1) * 16],
                                lhsT=hbf[:, j * 128 : (j + 1) * 128],
                                rhs=i16_sb[:],
                                start=True,
                                stop=True,
                            )
                        hT = hpool.tile([128, 128], bf16, name="hT")
                        nc.vector.tensor_copy(hT[:], psT[:])
                    else:
                        nc.scalar.dma_start(h_out[:], h_new[:])

                    # y = o * tanh(h')
                    th = wpool.tile([16, 1024], f32, name="th")
                    nc.scalar.activation(th[:], h_new[:], AF.Tanh)
                    y_sb = wpool.tile([16, 1024], f32, name="y_sb")
                    nc.vector.tensor_tensor(
                        out=y_sb[:], in0=sb_o[:], in1=th[:], op=ALU.mult
                    )
                    nc.scalar.dma_start(yout[t], y_sb[:])

                    h_prev = h_new

    nc.finalize()
    return nc


def _prep_host(inputs: dict):
    """Host-side pack. Returns dict of shared (replicated) arrays plus the
    per-core x slices."""
    import ml_dtypes

    f32 = np.float32
    bf = ml_dtypes.bfloat16
    x = np.ascontiguousarray(np.asarray(inputs["x"], dtype=np.int32))
    E = np.asarray(inputs["E"], dtype=f32)
    Wh = np.asarray(inputs["Wh"], dtype=f32)
    b = np.asarray(inputs["b"], dtype=f32)
    Ws = {g: np.asarray(inputs["W" + g], dtype=f32) for g in "fioc"}
    bs = {g: np.asarray(inputs["b" + g], dtype=f32) for g in "fioc"}

    WgP = np.concatenate([Ws[g] for g in "fioc"], axis=1)  # [1024, 4096]
    Wq = (Wh @ WgP).astype(f32)  # fold Wh into the gate weights
    wq_host = np.ascontiguousarray(
        Wq.reshape(KCH, 128, GN).transpose(1, 0, 2).reshape(128, KCH * GN)
    ).astype(bf)
    wg_host = np.ascontiguousarray(
        WgP.reshape(KCH, 128, GN).transpose(1, 0, 2).reshape(128, KCH * GN)
    ).astype(bf)
    E2 = (E + b[None, :]).astype(f32)
    eT_host = np.ascontiguousarray(
        E2.reshape(VCH, 128, KCH, 128).transpose(0, 3, 2, 1).reshape(VCH, 128, KCH * 128)
    ).astype(bf)
    bgP = np.concatenate([bs[g] for g in "fioc"]).astype(f32)  # [4096]
    bgb_host = np.ascontiguousarray(np.broadcast_to(bgP[None, :], (128, GN))).astype(
        f32
    )
    i16 = np.eye(16, dtype=f32).astype(bf)
    h0 = np.zeros((NB, UNITS), dtype=f32)

    shared = {
        "wq": wq_host,
        "wg": wg_host,
        "eT": eT_host,
        "bgb": bgb_host,
        "i16": i16,
        "h_in": h0,
    }
    return shared, x


def _make_exec(nc):
    """jit-compiled 8-core shard_map executor for a finalized Bacc module."""
    import jax
    from jax.sharding import Mesh, PartitionSpec
    from jax.experimental.shard_map import shard_map
    import concourse.mybir as mybir
    from concourse import bass2jax

    bass2jax.install_neuronx_cc_hook()

    in_names, out_names, out_avals, out_shapes = [], [], [], []
    for alloc in nc.m.functions[0].allocations:
        if not isinstance(alloc, mybir.MemoryLocationSet):
            continue
        name = alloc.memorylocations[0].name
        if alloc.kind == "ExternalInput":
            in_names.append(name)
        elif alloc.kind == "ExternalOutput":
            out_names.append(name)
            shape = tuple(alloc.tensor_shape)
            dtype = mybir.dt.np(alloc.dtype)
            out_avals.append(jax.core.ShapedArray(shape, dtype))
            out_shapes.append((shape, dtype))
    n_params = len(in_names)
    n_outs = len(out_avals)
    all_names = in_names + out_names

    def _body(*args):
        outs = bass2jax._bass_exec_p.bind(
            *args,
            out_avals=tuple(out_avals),
            in_names=tuple(all_names),
            out_names=tuple(out_names),
            lowering_input_output_aliases=(),
            sim_require_finite=True,
            sim_require_nnan=True,
            nc=nc,
        )
        return tuple(outs)

    devices = jax.devices()[:NCORES]
    mesh = Mesh(np.asarray(devices), ("core",))
    sharded = jax.jit(
        shard_map(
            _body,
            mesh=mesh,
            in_specs=(PartitionSpec("core"),) * (n_params + n_outs),
            out_specs=(PartitionSpec("core"),) * n_outs,
            check_rep=False,
        ),
        donate_argnums=tuple(range(n_params, n_params + n_outs)),
        keep_unused=True,
    )
    return sharded, in_names, out_names, out_shapes, mesh


_CACHE = {}


def _get_execs(chunks):
    execs = []
    for ci, s_chunk in enumerate(chunks):
        key = (s_chunk, ci > 0)
        if key not in _CACHE:
            nc = _build_chunk(s_chunk, with_table=(ci == 0), with_hin=(ci > 0))
            _CACHE[key] = _make_exec(nc)
        execs.append(_CACHE[key])
    return execs


def _chunk_sizes(s_steps):
    return [s_steps]


def _run(inputs: dict, s_steps: int = S, timing=None):
    import time

    import jax
    from jax.sharding import NamedSharding, PartitionSpec

    chunks = _chunk_sizes(s_steps)
    execs = _get_execs(chunks)
    shared, x = _prep_host(inputs)

    mesh = execs[0][4]
    sh = NamedSharding(mesh, PartitionSpec("core"))

    def put(arr):
        if isinstance(arr, list):
            cat = np.concatenate([np.asarray(a) for a in arr], axis=0)
        else:
            cat = np.concatenate([np.asarray(arr)] * NCORES, axis=0)
        return jax.device_put(cat, sh)

    staged = {k: put(v) for k, v in shared.items()}
    x = x[:, :s_steps]
    xs = []
    off = 0
    for s_chunk in chunks:
        xs.append(
            put([
                np.ascontiguousarray(x[c * NB : (c + 1) * NB, off : off + s_chunk])
                for c in range(NCORES)
            ])
        )
        off += s_chunk

    t0 = time.time()
    ys = []
    tab_dev = None
    h_dev = staged["h_in"]
    for ci, s_chunk in enumerate(chunks):
        sharded, in_names, out_names, out_shapes, _ = execs[ci]
        cur = dict(staged)
        cur["x"] = xs[ci]
        cur["h_in"] = h_dev
        if tab_dev is not None:
            cur["tab_in"] = tab_dev
        zeros = [
            jax.device_put(np.zeros((NCORES * sh0[0], *sh0[1:]), dt0), sh)
            for (sh0, dt0) in out_shapes
        ]
        args = [cur[n] for n in in_names] + zeros
        outs = sharded(*args)
        om = dict(zip(out_names, outs))
        ys.append(om["y"])
        h_dev = om["h_out"]
        if "tab" in om:
            tab_dev = om["tab"]
    jax.block_until_ready(ys + [h_dev])
    t1 = time.time()
    if timing is not None:
        timing.append(t1 - t0)

    out = np.empty((B, s_steps, UNITS), dtype=np.float32)
    off = 0
    for ci, s_chunk in enumerate(chunks):
        yc = np.asarray(ys[ci]).reshape(NCORES, s_chunk, NB, UNITS)
        for c in range(NCORES):
            out[c * NB : (c + 1) * NB, off : off + s_chunk, :] = yc[c].transpose(
                1, 0, 2
            )
        off += s_chunk
    return out


def kernel(**inputs) -> np.ndarray:
    return _run(inputs, S)


# revision 8
# speedup vs baseline: 203.9874x; 203.9874x over previous
"""DiscreteLSTM Trainium2 kernel — data-parallel over batch, zero collectives.

Reference math per step t:
    inp = h @ Wh + E[x_t] + b
    f,i,o = sigmoid(inp @ W{f,i,o} + b{f,i,o}); c = tanh(inp @ Wc + bc)
    h' = f*h + i*c ; y = o*tanh(h')

Folded form (exact up to fp reassociation):
    pre_g = h @ (Wh @ Wg) + T[x_t]   where T[v] = (E[v]+b) @ Wg + bg
so each step is 1 gate matmul + a row-gather from a precomputed per-token
gate table.

Sharding: pure data-parallel — core c owns batch rows [c*16,(c+1)*16) and
runs the full recurrence on them with replicated weights. No inter-core
communication at all, so the whole problem is one NEFF execution.

Precision: matmul operands are fp16; the folded weight Wq is applied as a
hi+lo fp16 pair (two matmul sets) because its quantization error is
amplified coherently through the 512-step recurrence (simulated final
rel-l2: fp16 Wq alone 3.9e-2, hi+lo 1.7e-3). h state is fp32, quantized
to fp16 only as the matmul operand. The 32000x4096 gate table is fp16,
built on-device (dense fp16 matmul over the vocab).

Per-step dataflow (per core):
  pre-gates accumulate in PSUM [48,2048] f32 (rows 0:16 = batch for f|i
  col-blocks, rows 32:48 for o|c — matmul out base-partition must be a
  multiple of 32; 4 gates fit in 4 PSUM banks). Gathered table rows seed
  PSUM via a 16x16-identity matmul. Pre-gates are copied to SBUF fp16 and
  transposed unit-major via 32 PE identity-matmuls, and ALL activations +
  elementwise run in transposed [128,*] packed tiles at full lane width.
  h' is born transposed, so the next step's stationary operand needs only
  an fp16 cast.
"""

import numpy as np

B = 128
S = 512
UNITS = 1024
VOCAB = 32000
NCORES = 8
NB = B // NCORES           # 16 batch rows per core
KCH = UNITS // 128         # 8 contraction chunks
VCH = VOCAB // 128         # 250 vocab chunks
GN = 4 * UNITS             # 4096 packed gate cols, order (f,i,o,c)

# gate -> (psum partition base, psum col base) in the [48, 2048] gates tile
GATE_POS = {"f": (0, 0), "i": (0, 1024), "o": (32, 0), "c": (32, 1024)}
GATE_COL = {"f": 0, "i": 1024, "o": 2048, "c": 3072}  # col block in table/wq
GATE_TCOL = {"f": 0, "i": 128, "c": 256, "o": 384}    # col block in psT/gact


def _build_chunk(s_steps: int, with_table: bool, with_hin: bool):
    """One NEFF: optionally build the gate table, then run s_steps of the
    recurrence on this core's 16 batch rows. Output y is transposed
    unit-major: y[t, p, j*16+m] = y_logical[m, t, j*128+p]."""
    import concourse.bass as bass
    import concourse.mybir as mybir
    import concourse.tile as tile
    from concourse import bacc

    f32 = mybir.dt.float32
    f16 = mybir.dt.float16
    i32 = mybir.dt.int32
    AF = mybir.ActivationFunctionType
    ALU = mybir.AluOpType

    nc = bacc.Bacc(
        "TRN2",
        target_bir_lowering=False,
        debug=False,
        num_devices=NCORES,
        enable_partition_id=False,
    )

    wq = nc.dram_tensor("wq", [128, KCH * GN], f16, kind="ExternalInput")
    wqlo = nc.dram_tensor("wqlo", [128, KCH * GN], f16, kind="ExternalInput")
    xin = nc.dram_tensor("x", [NB, s_steps], i32, kind="ExternalInput")
    i16in = nc.dram_tensor("i16", [48, 16], f16, kind="ExternalInput")
    if with_table:
        wg = nc.dram_tensor("wg", [128, KCH * GN], f16, kind="ExternalInput")
        eT = nc.dram_tensor("eT", [VCH, 128, KCH * 128], f16, kind="ExternalInput")
        bgb = nc.dram_tensor("bgb", [128, GN], f32, kind="ExternalInput")
        tab = nc.dram_tensor("tab", [VOCAB, GN], f16, kind="Internal")
    else:
        tab = nc.dram_tensor("tab_in", [VOCAB, GN], f16, kind="ExternalInput")
    if with_hin:
        h_in = nc.dram_tensor("h_in", [128, 128], f32, kind="ExternalInput")
    yout = nc.dram_tensor("y", [s_steps, 128, 128], f32, kind="ExternalOutput")
    h_out = nc.dram_tensor("h_out", [128, 128], f32, kind="ExternalOutput")

    with tile.TileContext(nc) as tc:
        with (
            tc.tile_pool(name="const", bufs=1) as cpool,
            tc.tile_pool(name="gbuf", bufs=3) as gpool,
            tc.tile_pool(name="htile", bufs=2) as hpool,
        ):
            # identity at partition bases 0 and 32 (matmul needs
            # lhsT/rhs on the same base partition)
            i16_sb = cpool.tile([48, 16], f16, name="i16_sb")
            nc.sync.dma_start(i16_sb[:], i16in[:])
            x_sb = cpool.tile([NB, s_steps], i32, name="x_sb")
            nc.sync.dma_start(x_sb[:], xin[:])

            if with_table:
                # ---------- phase 1: tab = (E+b) @ WgPack + bg ----------
                with (
                    tc.tile_pool(name="wgp", bufs=1) as wgpool,
                    tc.tile_pool(name="etile", bufs=3) as epool,
                    tc.tile_pool(name="tstage", bufs=3) as tpool,
                    tc.tile_pool(name="psum_t", bufs=2, space="PSUM") as ppt,
                ):
                    wg_sb = wgpool.tile([128, KCH * GN], f16, name="wg_sb")
                    nc.sync.dma_start(wg_sb[:], wg[:])
                    bgb_sb = wgpool.tile([128, GN], f32, name="bgb_sb")
                    nc.sync.dma_start(bgb_sb[:], bgb[:])
                    for v in range(VCH):
                        et = epool.tile([128, KCH * 128], f16, name="et")
                        nc.sync.dma_start(et[:], eT[v])
                        for half in range(2):
                            pt = ppt.tile([128, 2048], f32, space="PSUM", name="pt")
                            for k in range(KCH):
                                for q in range(4):
                                    nc.tensor.matmul(
                                        pt[:, q * 512 : (q + 1) * 512],
                                        lhsT=et[:, k * 128 : (k + 1) * 128],
                                        rhs=wg_sb[
                                            :,
                                            k * GN
                                            + half * 2048
                                            + q * 512 : k * GN
                                            + half * 2048
                                            + (q + 1) * 512,
                                        ],
                                        start=(k == 0),
                                        stop=(k == KCH - 1),
                                    )
                            ts = tpool.tile([128, 2048], f16, name="ts")
                            nc.vector.tensor_tensor(
                                out=ts[:],
                                in0=pt[:],
                                in1=bgb_sb[:, half * 2048 : (half + 1) * 2048],
                                op=ALU.add,
                            )
                            nc.gpsimd.dma_start(
                                tab[
                                    v * 128 : (v + 1) * 128,
                                    half * 2048 : (half + 1) * 2048,
                                ],
                                ts[:],
                            )

            # ---------- phase 2: recurrence ----------
            with (
                tc.tile_pool(name="wqp", bufs=1) as wqpool,
                tc.tile_pool(name="pgs", bufs=2) as pgpool,
                tc.tile_pool(name="gact", bufs=2) as gapool,
                tc.tile_pool(name="state", bufs=2) as spool,
                tc.tile_pool(name="psum_g", bufs=1, space="PSUM") as ppg,
                tc.tile_pool(name="psum_tr", bufs=2, space="PSUM") as ppr,
            ):
                wq_sb = wqpool.tile([128, KCH * GN], f16, name="wq_sb")
                nc.sync.dma_start(wq_sb[:], wq[:])
                wqlo_sb = wqpool.tile([128, KCH * GN], f16, name="wqlo_sb")
                nc.sync.dma_start(wqlo_sb[:], wqlo[:])

                # state tile cols: 0:128 hT(f32), 128:256 t1, 256:384 t2,
                #                  384:512 tanh(hT), 512:640 yT
                if with_hin:
                    state_prev = spool.tile([128, 640], f32, name="state")
                    nc.sync.dma_start(state_prev[:, 0:128], h_in[:])
                    hT16 = hpool.tile([128, 128], f16, name="hT16")
                    nc.vector.tensor_copy(hT16[:], state_prev[:, 0:128])
                else:
                    state_prev = None
                    hT16 = None

                for t in range(s_steps):
                    first = state_prev is None
                    g_sb = gpool.tile([NB, GN], f16, name="g_sb")
                    nc.gpsimd.indirect_dma_start(
                        out=g_sb[:],
                        out_offset=None,
                        in_=tab[:],
                        in_offset=bass.IndirectOffsetOnAxis(
                            ap=x_sb[:, t : t + 1], axis=0
                        ),
                    )

                    ps = ppg.tile([48, 2048], f32, space="PSUM", name="ps_gate")
                    pg = pgpool.tile([48, 2048], f16, name="pg")
                    psT = ppr.tile([128, 512], f32, space="PSUM", name="psT")
                    ga = gapool.tile([128, 512], f32, name="ga")
                    # per gate: seed + accumulate (hi then lo), then copy the
                    # finished pre-gate rows to SBUF fp16 and transpose on PE
                    for gname in "fico":
                        pb, cb = GATE_POS[gname]
                        wc = GATE_COL[gname]
                        for j in range(2):
                            reg = ps[pb : pb + 16, cb + j * 512 : cb + (j + 1) * 512]
                            nc.tensor.matmul(
                                reg,
                                lhsT=i16_sb[0:16, :],
                                rhs=g_sb[:, wc + j * 512 : wc + (j + 1) * 512],
                                start=True,
                                stop=first,
                            )
                            if not first:
                                for wsb, last in ((wq_sb, False), (wqlo_sb, True)):
                                    for k in range(KCH):
                                        nc.tensor.matmul(
                                            reg,
                                            lhsT=hT16[:, k * 16 : (k + 1) * 16],
                                            rhs=wsb[
                                                :,
                                                k * GN
                                                + wc
                                                + j * 512 : k * GN
                                                + wc
                                                + (j + 1) * 512,
                                            ],
                                            start=False,
                                            stop=(last and k == KCH - 1),
                                        )
                        # pre-gate rows -> SBUF fp16
                        nc.vector.tensor_copy(
                            pg[pb : pb + 16, cb : cb + 1024],
                            ps[pb : pb + 16, cb : cb + 1024],
                        )
                        # transpose to unit-major [128, 128] block of psT
                        tcol = GATE_TCOL[gname]
                        for j in range(KCH):
                            nc.tensor.matmul(
                                psT[:, tcol + j * 16 : tcol + (j + 1) * 16],
                                lhsT=pg[pb : pb + 16, cb + j * 128 : cb + (j + 1) * 128],
                                rhs=i16_sb[pb : pb + 16, :],
                                start=True,
                                stop=True,
                            )
                        if gname == "i":
                            # f,i ready: sigmoid over psT cols 0:256
                            nc.scalar.activation(
                                ga[:, 0:256], psT[:, 0:256], AF.Sigmoid
                            )
                        elif gname == "c":
                            nc.scalar.activation(
                                ga[:, 256:384], psT[:, 256:384], AF.Tanh
                            )
                        elif gname == "o":
                            nc.scalar.activation(
                                ga[:, 384:512], psT[:, 384:512], AF.Sigmoid
                            )

                    # h' = f*h + i*c  (all transposed [128,128] blocks)
                    state = spool.tile([128, 640], f32, name="state")
                    if first:
                        nc.vector.tensor_tensor(
                            out=state[:, 0:128], in0=ga[:, 128:256],
                            in1=ga[:, 256:384], op=ALU.mult,
                        )
                    else:
                        nc.vector.tensor_tensor(
                            out=state[:, 128:256], in0=ga[:, 0:128],
                            in1=state_prev[:, 0:128], op=ALU.mult,
                        )
                        nc.vector.tensor_tensor(
                            out=state[:, 256:384], in0=ga[:, 128:256],
                            in1=ga[:, 256:384], op=ALU.mult,
                        )
                        nc.vector.tensor_tensor(
                            out=state[:, 0:128], in0=state[:, 128:256],
                            in1=state[:, 256:384], op=ALU.add,
                        )

                    if t < s_steps - 1:
                        hT16 = hpool.tile([128, 128], f16, name="hT16")
                        nc.vector.tensor_copy(hT16[:], state[:, 0:128])
                    else:
                        nc.scalar.dma_start(h_out[:], state[:, 0:128])

                    # y = o * tanh(h')
                    nc.scalar.activation(
                        state[:, 384:512], state[:, 0:128], AF.Tanh
                    )
                    nc.vector.tensor_tensor(
                        out=state[:, 512:640], in0=ga[:, 384:512],
                        in1=state[:, 384:512], op=ALU.mult,
                    )
                    nc.scalar.dma_start(yout[t], state[:, 512:640])

                    state_prev = state

    nc.finalize()
    return nc


def _prep_host(inputs: dict):
    """Host-side pack. Returns dict of shared (replicated) arrays plus the
    per-core x slices."""
    f32 = np.float32
    f16 = np.float16
    x = np.ascontiguousarray(np.asarray(inputs["x"], dtype=np.int32))
    E = np.asarray(inputs["E"], dtype=f32)
    Wh = np.asarray(inputs["Wh"], dtype=f32)
    b = np.asarray(inputs["b"], dtype=f32)
    Ws = {g: np.asarray(inputs["W" + g], dtype=f32) for g in "fioc"}
    bs = {g: np.asarray(inputs["b" + g], dtype=f32) for g in "fioc"}

    WgP = np.concatenate([Ws[g] for g in "fioc"], axis=1)  # [1024, 4096]
    Wq = (Wh @ WgP).astype(f32)  # fold Wh into the gate weights
    Wq_pack = np.ascontiguousarray(
        Wq.reshape(KCH, 128, GN).transpose(1, 0, 2).reshape(128, KCH * GN)
    )
    wq_host = Wq_pack.astype(f16)
    wqlo_host = (Wq_pack - wq_host.astype(f32)).astype(f16)
    wg_host = np.ascontiguousarray(
        WgP.reshape(KCH, 128, GN).transpose(1, 0, 2).reshape(128, KCH * GN)
    ).astype(f16)
    E2 = (E + b[None, :]).astype(f32)
    eT_host = np.ascontiguousarray(
        E2.reshape(VCH, 128, KCH, 128).transpose(0, 3, 2, 1).reshape(VCH, 128, KCH * 128)
    ).astype(f16)
    bgP = np.concatenate([bs[g] for g in "fioc"]).astype(f32)  # [4096]
    bgb_host = np.ascontiguousarray(np.broadcast_to(bgP[None, :], (128, GN))).astype(
        f32
    )
    i16 = np.zeros((48, 16), dtype=f16)
    i16[0:16] = np.eye(16, dtype=f16)
    i16[32:48] = np.eye(16, dtype=f16)
    h0 = np.zeros((128, 128), dtype=f32)

    shared = {
        "wq": wq_host,
        "wqlo": wqlo_host,
        "wg": wg_host,
        "eT": eT_host,
        "bgb": bgb_host,
        "i16": i16,
        "h_in": h0,
    }
    return shared, x


def _make_exec(nc):
    """jit-compiled 8-core shard_map executor for a finalized Bacc module."""
    import jax
    from jax.sharding import Mesh, PartitionSpec
    from jax.experimental.shard_map import shard_map
    import concourse.mybir as mybir
    from concourse import bass2jax

    bass2jax.install_neuronx_cc_hook()

    in_names, out_names, out_avals, out_shapes = [], [], [], []
    for alloc in nc.m.functions[0].allocations:
        if not isinstance(alloc, mybir.MemoryLocationSet):
            continue
        name = alloc.memorylocations[0].name
        if alloc.kind == "ExternalInput":
            in_names.append(name)
        elif alloc.kind == "ExternalOutput":
            out_names.append(name)
            shape = tuple(alloc.tensor_shape)
            dtype = mybir.dt.np(alloc.dtype)
            out_avals.append(jax.core.ShapedArray(shape, dtype))
            out_shapes.append((shape, dtype))
    n_params = len(in_names)
    n_outs = len(out_avals)
    all_names = in_names + out_names

    def _body(*args):
        outs = bass2jax._bass_exec_p.bind(
            *args,
            out_avals=tuple(out_avals),
            in_names=tuple(all_names),
            out_names=tuple(out_names),
            lowering_input_output_aliases=(),
            sim_require_finite=True,
            sim_require_nnan=True,
            nc=nc,
        )
        return tuple(outs)

    devices = jax.devices()[:NCORES]
    mesh = Mesh(np.asarray(devices), ("core",))
    sharded = jax.jit(
        shard_map(
            _body,
            mesh=mesh,
            in_specs=(PartitionSpec("core"),) * (n_params + n_outs),
            out_specs=(PartitionSpec("core"),) * n_outs,
            check_rep=False,
        ),
        donate_argnums=tuple(range(n_params, n_params + n_outs)),
        keep_unused=True,
    )
    return sharded, in_names, out_names, out_shapes, mesh


_CACHE = {}


def _get_execs(chunks):
    execs = []
    for ci, s_chunk in enumerate(chunks):
        key = (s_chunk, ci > 0)
        if key not in _CACHE:
            nc = _build_chunk(s_chunk, with_table=(ci == 0), with_hin=(ci > 0))
            _CACHE[key] = _make_exec(nc)
        execs.append(_CACHE[key])
    return execs


def _chunk_sizes(s_steps):
    return [s_steps]


def _run(inputs: dict, s_steps: int = S, timing=None):
    import time

    import jax
    from jax.sharding import NamedSharding, PartitionSpec

    chunks = _chunk_sizes(s_steps)
    execs = _get_execs(chunks)
    shared, x = _prep_host(inputs)

    mesh = execs[0][4]
    sh = NamedSharding(mesh, PartitionSpec("core"))

    def put(arr):
        if isinstance(arr, list):
            cat = np.concatenate([np.asarray(a) for a in arr], axis=0)
        else:
            cat = np.concatenate([np.asarray(arr)] * NCORES, axis=0)
        return jax.device_put(cat, sh)

    staged = {k: put(v) for k, v in shared.items()}
    x = x[:, :s_steps]
    xs = []
    off = 0
    for s_chunk in chunks:
        xs.append(
            put([
                np.ascontiguousarray(x[c * NB : (c + 1) * NB, off : off + s_chunk])
                for c in range(NCORES)
            ])
        )
        off += s_chunk

    # pre-allocate donated output buffers on-device (outside the timed span;
    # their contents are irrelevant — every output element is written)
    import jax.numpy as jnp

    zeros_per_chunk = []
    for ci in range(len(chunks)):
        out_shapes = execs[ci][3]
        mk = jax.jit(
            lambda shapes=tuple(out_shapes): tuple(
                jnp.zeros((NCORES * s[0], *s[1:]), d) for (s, d) in shapes
            ),
            out_shardings=tuple(sh for _ in out_shapes),
        )
        zeros_per_chunk.append(list(mk()))
    jax.block_until_ready(zeros_per_chunk)

    t0 = time.time()
    ys = []
    tab_dev = None
    h_dev = staged["h_in"]
    for ci, s_chunk in enumerate(chunks):
        sharded, in_names, out_names, out_shapes, _ = execs[ci]
        cur = dict(staged)
        cur["x"] = xs[ci]
        cur["h_in"] = h_dev
        if tab_dev is not None:
            cur["tab_in"] = tab_dev
        args = [cur[n] for n in in_names] + zeros_per_chunk[ci]
        outs = sharded(*args)
        om = dict(zip(out_names, outs))
        ys.append(om["y"])
        h_dev = om["h_out"]
        if "tab" in om:
            tab_dev = om["tab"]
    jax.block_until_ready(ys + [h_dev])
    t1 = time.time()
    if timing is not None:
        timing.append(t1 - t0)

    out = np.empty((B, s_steps, UNITS), dtype=np.float32)
    off = 0
    for ci, s_chunk in enumerate(chunks):
        # y[t, p, j*16+m] = y_logical[m, t, j*128+p] per core
        yc = np.asarray(ys[ci]).reshape(NCORES, s_chunk, 128, KCH, NB)
        for c in range(NCORES):
            out[c * NB : (c + 1) * NB, off : off + s_chunk, :] = (
                yc[c].transpose(3, 0, 2, 1).reshape(NB, s_chunk, UNITS)
            )
        off += s_chunk
    return out


def kernel(**inputs) -> np.ndarray:
    return _run(inputs, S)
